# revision 24
# baseline (speedup 1.0000x reference)
"""Trainium2 Bass kernel for AngularAwareTemporalAttention (baseline copy).

See kernel.py for the full description; this is the known-good 702us
baseline preserved for A/B testing.
"""

import os
import numpy as np
import ml_dtypes

import concourse.bass as bass
import concourse.mybir as mybir
import concourse.tile as tile
from concourse import bacc
from concourse.bass_utils import run_bass_kernel_spmd
from concourse.masks import make_identity

B, N, T, D = 4, 64, 128, 1024
H, HD = 16, 64
SCALE = HD ** -0.5
BN = B * N
NCORES = 8
S_PER_CORE = BN // NCORES      # 32 sequences per core
R = S_PER_CORE * T             # 4096 rows per core
SB = 4                         # sequences per block
RB = SB * T                    # 512 rows per block
NBLK = S_PER_CORE // SB        # 8 blocks
KC = D // 128                  # 8 contraction chunks of 128
BF16 = mybir.dt.bfloat16
F32 = mybir.dt.float32

_CACHE = {}
LAST_RESULT = None


def _build():
    nc = bacc.Bacc()
    xt = nc.declare_dram_parameter("xt", [128, KC, R], BF16, isOutput=False)
    wqk = nc.declare_dram_parameter("wqk", [128, KC, 2 * D], BF16, isOutput=False)
    wv = nc.declare_dram_parameter("wv", [128, KC, D], BF16, isOutput=False)
    wp = nc.declare_dram_parameter("wp", [128, KC, D], BF16, isOutput=False)
    bvec = nc.declare_dram_parameter("bvec", [128, 3], F32, isOutput=False)
    sc8 = nc.declare_dram_parameter("sc8", [128, 1], F32, isOutput=False)
    out = nc.declare_dram_parameter("out", [R, D], F32, isOutput=True)

    with tile.TileContext(nc) as tc:
        with (
            tc.tile_pool(name="consts", bufs=1) as consts,
            tc.tile_pool(name="wpool", bufs=1) as wpool,
            tc.tile_pool(name="xpool", bufs=2) as xpool,
            tc.tile_pool(name="qkpool", bufs=2) as qkpool,
            tc.tile_pool(name="vpool", bufs=2) as vpool,
            tc.tile_pool(name="aopool", bufs=2) as aopool,
            tc.tile_pool(name="opool", bufs=3) as opool,
            tc.tile_pool(name="spool", bufs=4) as spool,
            tc.tile_pool(name="rpool", bufs=4) as rpool,
            tc.tile_pool(name="ppbig", bufs=2, space="PSUM") as pp_big,
            tc.tile_pool(name="pplog", bufs=2, space="PSUM") as pp_log,
            tc.tile_pool(name="pppv", bufs=2, space="PSUM") as pp_pv,
            tc.tile_pool(name="ppden", bufs=2, space="PSUM") as pp_den,
        ):
            xt0 = xpool.tile([128, KC, RB], BF16, tag="xt", name="xt_0")
            w_qk = wpool.tile([128, KC, 2 * D], BF16)
            w_v = wpool.tile([128, KC, D], BF16)
            w_p = wpool.tile([128, KC, D], BF16)
            for kc in range(KC):
                nc.sync.dma_start(xt0[:, kc, :], xt[:, kc, 0:RB])
                nc.sync.dma_start(w_qk[:, kc, :], wqk[:, kc, :])
            for kc in range(KC):
                nc.sync.dma_start(w_v[:, kc, :], wv[:, kc, :])
            for kc in range(KC):
                nc.sync.dma_start(w_p[:, kc, :], wp[:, kc, :])

            ones_sb = consts.tile([128, 1], BF16)
            nc.vector.memset(ones_sb[:], 1.0)
            ident = consts.tile([128, 128], F32)
            make_identity(nc, ident[:])
            ident_bf = consts.tile([128, 128], BF16)
            nc.vector.tensor_copy(ident_bf[:], ident[:])
            sc8_sb = consts.tile([128, 1], F32)
            nc.sync.dma_start(sc8_sb[:], sc8[:])

            bv_sb = consts.tile([128, 3], F32)
            nc.sync.dma_start(bv_sb[:], bvec[:])
            sq = consts.tile([128, 3], F32)
            nc.vector.tensor_mul(sq[:], bv_sb[:], bv_sb[:])
            ssq = consts.tile([128, 1], F32)
            nc.vector.reduce_sum(ssq[:], sq[:], axis=mybir.AxisListType.X)
            nrm = consts.tile([128, 1], F32)
            nc.scalar.sqrt(nrm[:], ssq[:])
            nc.vector.tensor_scalar_add(nrm[:], nrm[:], 1e-6)
            rinv = consts.tile([128, 1], F32)
            nc.vector.reciprocal(rinv[:], nrm[:])
            bn = consts.tile([128, 3], F32)
            nc.vector.tensor_scalar_mul(bn[:], bv_sb[:], rinv[:])
            pt = pp_log.tile([128, 128], F32, tag="log")
            nc.tensor.transpose(pt[:3, :], bn[:], ident[:])
            bnT = consts.tile([3, 128], F32)
            nc.vector.tensor_copy(bnT[:], pt[:3, :])
            cosp = pp_log.tile([128, 128], F32, tag="log")
            nc.tensor.matmul(cosp[:], bnT[:], bnT[:], start=True, stop=True)
            bias_rep = consts.tile([128, 4 * T], F32)
            for rep in range(4):
                nc.vector.tensor_scalar(
                    out=bias_rep[:, rep * T:(rep + 1) * T], in0=cosp[:],
                    scalar1=1.0, scalar2=-1.0,
                    op0=mybir.AluOpType.min, op1=mybir.AluOpType.max)
            nc.vector.tensor_scalar_mul(bias_rep[:], bias_rep[:], sc8_sb[:])
            bias_bf = consts.tile([128, 4 * T], BF16)
            nc.vector.tensor_copy(bias_bf[:], bias_rep[:])

            def qk_unit(xt_blk, qkT, fc):
                ps = pp_big.tile([128, RB], F32, tag="gemm")
                for kc in range(KC):
                    nc.tensor.matmul(
                        ps[:], w_qk[:, kc, fc * 128:(fc + 1) * 128],
                        xt_blk[:, kc, :],
                        start=(kc == 0), stop=(kc == KC - 1))
                nc.vector.tensor_copy(qkT[:, fc, :], ps[:])

            def v_unit(xt_blk, v_blk, rc, nf):
                ps = pp_big.tile([128, RB], F32, tag="gemm")
                for kc in range(KC):
                    nc.tensor.matmul(
                        ps[:], xt_blk[:, kc, rc * 128:(rc + 1) * 128],
                        w_v[:, kc, nf * 512:(nf + 1) * 512],
                        start=(kc == 0), stop=(kc == KC - 1))
                nc.vector.tensor_copy(
                    v_blk[:, rc, nf * 8:(nf + 1) * 8, 0:64],
                    ps[:].rearrange("p (h d) -> p h d", d=64))

            def attn_unit(qkT, v_blk, aoT, s, g):
                po = (g % 2) * 64
                fbase = (g // 2) * 4
                lp = pp_log.tile([128, 4 * T], F32, tag="log")
                for hh in range(4):
                    fcq = fbase + hh
                    sl = slice(hh * T, (hh + 1) * T)
                    nc.tensor.matmul(lp[:, sl], ident_bf[:], bias_bf[:, sl],
                                     start=True, stop=False)
                    nc.tensor.matmul(
                        lp[:, sl],
                        qkT[po:po + 64, 8 + fcq, s * T:(s + 1) * T],
                        qkT[po:po + 64, fcq, s * T:(s + 1) * T],
                        start=False, stop=True)
                st = spool.tile([128, 4 * T], BF16, tag="st")
                nc.scalar.activation(
                    st[:], lp[:], mybir.ActivationFunctionType.Exp,
                    scale=SCALE)
                po_ps = pp_pv.tile([128, 4, 65], F32, tag="pv")
                for hh in range(4):
                    h = 2 * (fbase + hh) + (g % 2)
                    nc.tensor.matmul(
                        po_ps[:, hh, 0:65],
                        st[:, hh * T:(hh + 1) * T],
                        v_blk[:, s, h, 0:65],
                        start=True, stop=True)
                rec_col = rpool.tile([128, 4], F32, tag="rec")
                nc.vector.reciprocal(rec_col[:], po_ps[:, :, 64])
                ao_nat = spool.tile([128, 4, 64], BF16, tag="aonat")
                for hh in range(4):
                    nc.vector.tensor_scalar_mul(
                        ao_nat[:, hh, :], po_ps[:, hh, 0:64],
                        rec_col[:, hh:hh + 1])
                tp = pp_den.tile([64, 4, T], BF16, tag="tp")
                for hh in range(4):
                    nc.tensor.transpose(
                        tp[:, hh, :], ao_nat[:, hh, :], ident_bf[:])
                nc.vector.tensor_copy(
                    aoT[po:po + 64, fbase:fbase + 4, s * T:(s + 1) * T],
                    tp[:])

            def proj_unit(aoT, r0, rc):
                orow = opool.tile([128, D], F32, tag="orow")
                for nf in range(2):
                    ps = pp_big.tile([128, RB], F32, tag="gemm")
                    for kc in range(KC):
                        nc.tensor.matmul(
                            ps[:], aoT[:, kc, rc * 128:(rc + 1) * 128],
                            w_p[:, kc, nf * 512:(nf + 1) * 512],
                            start=(kc == 0), stop=(kc == KC - 1))
                    nc.vector.tensor_copy(
                        orow[:, nf * 512:(nf + 1) * 512], ps[:])
                nc.sync.dma_start(
                    out[r0 + rc * 128: r0 + (rc + 1) * 128, :], orow[:])

            prev = None
            for b in range(NBLK + 1):
                cur = None
                if b < NBLK:
                    if b == 0:
                        xt_blk = xt0
                    else:
                        xt_blk = xpool.tile([128, KC, RB], BF16, tag="xt")
                        nc.sync.dma_start(xt_blk[:],
                                          xt[:, :, b * RB:(b + 1) * RB])
                    v_blk = vpool.tile([128, SB, 16, 65], BF16, tag="v",
                                       name=f"v_{b}")
                    nc.vector.memset(v_blk[:, :, :, 64:65], 1.0)
                    cur = {
                        "xt": xt_blk,
                        "qkT": qkpool.tile([128, 16, RB], BF16, tag="qkT",
                                           name=f"qkT_{b}"),
                        "v": v_blk,
                    }
                if prev is not None:
                    prev["aoT"] = aopool.tile([128, KC, RB], BF16, tag="aoT",
                                              name=f"aoT_{b}")

                for i in range(16):
                    if cur is not None:
                        qk_unit(cur["xt"], cur["qkT"], i)
                    if prev is not None:
                        attn_unit(prev["qkT"], prev["v"], prev["aoT"],
                                  i // 4, i % 4)
                        # epilogue block: no QK GEMMs to keep the PE dense,
                        # so interleave each sequence's projection GEMMs
                        # into the attention stream (one unit late, so the
                        # aoT transpose copies have landed)
                        if cur is None and i >= 5 and (i - 1) % 4 == 0:
                            proj_unit(prev["aoT"], (b - 1) * RB,
                                      (i - 1) // 4 - 1)
                for i in range(8):
                    if cur is not None:
                        v_unit(cur["xt"], cur["v"], i // 2, i % 2)
                        if prev is not None and i % 2 == 1:
                            proj_unit(prev["aoT"], (b - 1) * RB, i // 2)
                    elif prev is not None and i == 0:
                        proj_unit(prev["aoT"], (b - 1) * RB, 3)
                prev = cur
    nc.finalize()
    return nc


def kernel(**inputs):
    global LAST_RESULT
    x = np.ascontiguousarray(np.asarray(inputs["x"], dtype=np.float32))
    bvecs = np.ascontiguousarray(np.asarray(inputs["bvecs"], dtype=np.float32))
    qkv_w = np.asarray(inputs["qkv_w"], dtype=np.float32)
    qkv_b = np.asarray(inputs["qkv_b"], dtype=np.float32)
    proj_w = np.asarray(inputs["proj_w"], dtype=np.float32)
    proj_b = np.asarray(inputs["proj_b"], dtype=np.float32)
    s_ab = float(np.asarray(inputs["angular_bias_scale"], dtype=np.float32).reshape(-1)[0])

    bf = ml_dtypes.bfloat16
    wqk_p = np.ascontiguousarray(
        qkv_w[:, :2 * D].reshape(KC, 128, 2 * D).transpose(1, 0, 2)).astype(bf)
    wv_p = np.ascontiguousarray(
        qkv_w[:, 2 * D:3 * D].reshape(KC, 128, D).transpose(1, 0, 2)).astype(bf)
    wp_p = np.ascontiguousarray(
        proj_w.reshape(KC, 128, D).transpose(1, 0, 2)).astype(bf)
    sc8_arr = np.full((128, 1), s_ab * 8.0, dtype=np.float32)

    in_maps = []
    for c in range(NCORES):
        xs = x[c * S_PER_CORE:(c + 1) * S_PER_CORE].reshape(R, D)
        xt_p = np.ascontiguousarray(
            xs.T.reshape(KC, 128, R).transpose(1, 0, 2)).astype(bf)
        in_maps.append({
            "xt": xt_p,
            "wqk": wqk_p,
            "wv": wv_p,
            "wp": wp_p,
            "bvec": np.ascontiguousarray(bvecs[(c * S_PER_CORE) // N]),
            "sc8": sc8_arr,
        })

    if "nc" not in _CACHE:
        _CACHE["nc"] = _build()
    nc = _CACHE["nc"]

    last_err = None
    for attempt in range(3):
        try:
            res = run_bass_kernel_spmd(nc, in_maps, core_ids=list(range(NCORES)))
            outs = [np.asarray(res.results[i]["out"], dtype=np.float32)
                    for i in range(NCORES)]
            break
        except Exception as e:  # axon transfers are occasionally flaky
            last_err = e
            if attempt == 2:
                raise
    LAST_RESULT = res
    full = np.concatenate(outs, axis=0).reshape(BN, T, D)

    full = full + (qkv_b[2 * D:3 * D] @ proj_w + proj_b)[None, None, :]
    return full.astype(np.float32)
